# revision 8
# baseline (speedup 1.0000x reference)
"""Trainium2 Bass kernel for a small decoder block (nn_Decoder_75849122448079).

Math (N=4096 seq, W=512 width, P=64 proj, H=8 heads, F=2048 ffn):
  masked_mh = softmax(q_m k_m^T / 8) v_m @ w_o_sum      (w_o_sum = sum of H row-blocks of w_o)
  mh        = softmax(q_c k_c^T / 8) v_c @ w_o_sum      (q_c from masked_mh; k_c/v_c from x)
  h   = LN(mh + x) * g + b
  y   = LeakyReLU(h @ w1 + b1) @ w2 + b2
  out = LN(y + h) * g + b

Sharding: data-parallel over sequence rows. Each of the 8 cores computes 512
query rows end-to-end; K/V projections over the full sequence are computed
redundantly per core (no collectives). The host only slices inputs and
concatenates outputs.

Attention trick: scores are built transposed (S^T[k, q] = K Q^T), exp'd on the
ACT engine straight out of PSUM, and the softmax denominator rides along as a
ones-column appended to V, so no partition-axis reduction is ever needed.
"""

import os

import numpy as np

import concourse.bass as bass
import concourse.bacc as bacc
import concourse.mybir as mybir
import concourse.tile as tile
from concourse.bass_utils import run_bass_kernel_spmd
from concourse.masks import make_identity

N, W, P, H, F = 4096, 512, 64, 8, 2048
NCORES = 8
R = N // NCORES          # 512 rows per core
RT = R // 128            # 4 row tiles per core
WC = W // 128            # 4 contraction chunks over width
ST = N // 128            # 32 sequence (key) tiles
FC = F // 128            # 16 ffn-hidden tiles
FG = 4                   # ffn tiles per group (psum partial-accum group)
EPS = 1e-5
LEAKY = 0.01
SCALE = 0.125            # 1/sqrt(P)

f32 = mybir.dt.float32
f32r = mybir.dt.float32r
bf16 = mybir.dt.bfloat16

# Compute dtype mode: "f32" (exact, 4 cyc/row), "f32r" (fp32 storage, fast PE
# path), "bf16" (half storage, fast PE path, ~1e-3 rel err).
MODE = os.environ.get("BASS_DECODER_MODE", "f32")


def build_nc(mode=MODE):
    cd = bf16 if mode == "bf16" else f32     # storage dtype of PE-facing tiles
    nc = bacc.Bacc()

    names_2d = [("x_full", [N, W]), ("x_rows", [R, W]),
                ("w_q_m", [W, P]), ("w_k_m", [W, P]), ("w_v_m", [W, P]),
                ("w_q_c", [W, P]), ("w_k_c", [W, P]), ("w_v_c", [W, P]),
                ("w_o", [H * P, W]), ("ffn_w1", [W, F]), ("ffn_w2", [F, W])]
    names_1d = [("ln_g", [W]), ("ln_b", [W]), ("ffn_b1", [F]), ("ffn_b2", [W])]
    t = {}
    for n, s in names_2d + names_1d:
        t[n] = nc.declare_dram_parameter(n, s, f32, isOutput=False)
    t["out"] = nc.declare_dram_parameter("out", [R, W], f32, isOutput=True)

    with tile.TileContext(nc) as tc:
        _build(tc, mode, cd, t)
    return nc


def _row_bcast(ap, parts=128):
    """AP reading a 1-D DRAM tensor replicated across `parts` partitions."""
    a = ap[:]
    return bass.AP(tensor=a.tensor, offset=a.offset, ap=[[0, parts]] + list(a.ap))


def _build(tc, mode, cd, t):
    nc = tc.nc
    r = mode == "f32r"

    def bc(ap):  # matmul-input view
        return ap.bitcast(f32r) if r else ap

    def mm(out, lhsT, rhs, start=True, stop=True):
        nc.tensor.matmul(out, bc(lhsT), bc(rhs), start=start, stop=stop)

    def tp(out, in_, ident):  # PE transpose; out dtype must match in_ dtype
        nc.tensor.matmul(out.bitcast(f32r) if r else out, bc(in_), bc(ident),
                         is_transpose=True)

    # ------------------------------------------------------------------ pools
    from contextlib import ExitStack
    ctx = ExitStack()
    persist = ctx.enter_context(tc.tile_pool(name="persist", bufs=1))
    stream = ctx.enter_context(tc.tile_pool(name="stream", bufs=2))
    wstream = ctx.enter_context(tc.tile_pool(name="wstream", bufs=2))
    small = ctx.enter_context(tc.tile_pool(name="small", bufs=4))
    pt_pool = ctx.enter_context(tc.tile_pool(name="pt_pool", bufs=2))
    ps_big = ctx.enter_context(tc.tile_pool(name="ps_big", bufs=2, space="PSUM"))
    ps_kv = ctx.enter_context(tc.tile_pool(name="ps_kv", bufs=4, space="PSUM"))

    def big(shape, dtype=f32):
        return ps_big.tile(shape, dtype, tag="big", name="bigtile")

    def kv(shape, dtype=f32):
        return ps_kv.tile(shape, dtype, tag="kv", name="kvtile")

    # --------------------------------------------------------- constants
    ident = persist.tile([128, 128], cd)
    make_identity(nc, ident)
    if cd == f32:
        ident_f32 = ident
    else:
        ident_f32 = persist.tile([128, 128], f32)
        make_identity(nc, ident_f32)

    eps_t = persist.tile([128, 1], f32)
    nc.vector.memset(eps_t, EPS)

    g_rep = persist.tile([128, W], f32)
    nc.sync.dma_start(out=g_rep, in_=_row_bcast(t["ln_g"]))
    b_rep = persist.tile([128, W], f32)
    nc.sync.dma_start(out=b_rep, in_=_row_bcast(t["ln_b"]))
    b2_rep = persist.tile([128, W], f32)
    nc.sync.dma_start(out=b2_rep, in_=_row_bcast(t["ffn_b2"]))
    b1_sb = persist.tile([128, FC], f32)
    nc.sync.dma_start(out=b1_sb, in_=t["ffn_b1"].rearrange("(c p) -> p c", p=128))

    # qkv weights as [128, WC, P] (width chunk on partitions)
    def load_w(name):
        w = persist.tile([128, WC, P], cd, tag=f"w_{name}", name=f"w_{name}")
        if cd == f32:
            nc.sync.dma_start(out=w, in_=t[name].rearrange("(c p) d -> p c d", p=128))
        else:
            tmp = stream.tile([128, WC, P], f32, tag="wtmp")
            nc.sync.dma_start(out=tmp, in_=t[name].rearrange("(c p) d -> p c d", p=128))
            nc.vector.tensor_copy(w, tmp)
        return w

    wqm, wkm, wvm = load_w("w_q_m"), load_w("w_k_m"), load_w("w_v_m")
    wqc, wkc, wvc = load_w("w_q_c"), load_w("w_k_c"), load_w("w_v_c")

    # w_o_sum[d, w] = sum_h w_o[h*P + d, w]   -> [64, W]
    wo_re = t["w_o"].rearrange("(g u p) w -> p g u w", g=4, u=2)
    wos_f32 = persist.tile([64, W], f32)
    for g in range(4):
        wtmp = stream.tile([64, 2, W], f32, tag="xin")
        nc.sync.dma_start(out=wtmp, in_=wo_re[:, g, :, :])
        if g == 0:
            nc.vector.tensor_add(wos_f32, wtmp[:, 0, :], wtmp[:, 1, :])
        else:
            nc.vector.tensor_add(wos_f32, wos_f32, wtmp[:, 0, :])
            nc.vector.tensor_add(wos_f32, wos_f32, wtmp[:, 1, :])
    if cd == f32:
        wosum = wos_f32
    else:
        wosum = persist.tile([64, W], cd)
        nc.vector.tensor_copy(wosum, wos_f32)

    # ------------------------------------------------ x_rows (residual + Q^T)
    xr_nat = persist.tile([128, RT, W], f32)
    nc.sync.dma_start(out=xr_nat, in_=t["x_rows"].rearrange("(q p) w -> p q w", p=128))
    if cd == f32:
        xr_cd = xr_nat
    else:
        xr_cd = persist.tile([128, RT, W], cd)
        nc.vector.tensor_copy(xr_cd, xr_nat)

    # x_rows^T [128, WC, R]: xrT[p, c, q] = x_rows[q, c*128+p]
    xrT = persist.tile([128, WC, R], cd)
    for qt in range(RT):
        pst = big([128, WC, 128], cd)
        for wc in range(WC):
            tp(pst[:, wc, :], xr_cd[:, qt, wc * 128:(wc + 1) * 128], ident)
        nc.vector.tensor_copy(xrT[:, :, qt * 128:(qt + 1) * 128], pst)

    # generic  [64, n] = w^T @ rhsT  (contract over width chunks)
    def proj_T(wt, rhsT, n_free, tag):
        ps = kv([64, n_free])
        for wc in range(WC):
            mm(ps, wt[:, wc, :], rhsT[:, wc, :], start=(wc == 0), stop=(wc == WC - 1))
        sb = persist.tile([64, n_free], cd, tag=tag, name=tag)
        nc.vector.tensor_copy(sb, ps)
        return sb

    qmT = proj_T(wqm, xrT, R, "qmT")

    # ---------------------------------- stream x_full: K^T and V projections
    kmT = persist.tile([64, N], cd)
    kcT = persist.tile([64, N], cd)
    vm = persist.tile([128, ST, P + 1], cd)
    vc = persist.tile([128, ST, P + 1], cd)
    nc.vector.memset(vm[:, :, P:P + 1], 1.0)
    nc.vector.memset(vc[:, :, P:P + 1], 1.0)

    x_re = t["x_full"].rearrange("(s u p) w -> s u p w", p=128, u=2)  # [16,2,128,W]

    for pr in range(ST // 2):  # pairs of seq tiles
        xin = stream.tile([128, 2, W], f32, tag="xin")
        nc.sync.dma_start(out=xin, in_=x_re[pr])
        if cd == f32:
            xin_cd = xin
        else:
            xin_cd = stream.tile([128, 2, W], cd, tag="xin_cd")
            nc.vector.tensor_copy(xin_cd, xin)
        # transpose the pair: xT2[p, wc, u, s] = x[(2pr+u)*128+s, wc*128+p]
        pst = big([128, WC, 2, 128], cd)
        for u in range(2):
            for wc in range(WC):
                tp(pst[:, wc, u, :], xin_cd[:, u, wc * 128:(wc + 1) * 128], ident)
        xT2 = stream.tile([128, WC, 2, 128], cd, tag="xT2")
        nc.vector.tensor_copy(xT2, pst)

        # K^T blocks [64, 256]
        ps_km = kv([64, 2, 128])
        ps_kc = kv([64, 2, 128])
        for wc in range(WC):
            mm(ps_km, wkm[:, wc, :], xT2[:, wc, :, :],
               start=(wc == 0), stop=(wc == WC - 1))
        for wc in range(WC):
            mm(ps_kc, wkc[:, wc, :], xT2[:, wc, :, :],
               start=(wc == 0), stop=(wc == WC - 1))
        nc.vector.tensor_copy(kmT[:, pr * 256:(pr + 1) * 256], ps_km)
        nc.vector.tensor_copy(kcT[:, pr * 256:(pr + 1) * 256], ps_kc)

        # V natural blocks [128, 64]
        for u in range(2):
            st = 2 * pr + u
            ps_vm = kv([128, P])
            for wc in range(WC):
                mm(ps_vm, xT2[:, wc, u, :], wvm[:, wc, :],
                   start=(wc == 0), stop=(wc == WC - 1))
            nc.vector.tensor_copy(vm[:, st, 0:P], ps_vm)
            ps_vc = kv([128, P])
            for wc in range(WC):
                mm(ps_vc, xT2[:, wc, u, :], wvc[:, wc, :],
                   start=(wc == 0), stop=(wc == WC - 1))
            nc.vector.tensor_copy(vc[:, st, 0:P], ps_vc)

    # ------------------------------------------------------------- attention
    def attention(kT, v, qT, out_name):
        """A'^T = [v | 1]^T softmax_unnorm(qk^T/8)^T  -> [P+1, R] unnormalized."""
        ps_aT = kv([P + 1, R])
        G = ST // 2

        def scores(g):
            sT = big([128, 2, 512])
            for j in range(2):
                kt = g * 2 + j
                mm(sT[:, j, :], kT[:, kt * 128:(kt + 1) * 128], qT)
            return sT

        sT_prev = scores(0)
        for g in range(1, G + 1):
            sT_next = scores(g) if g < G else None
            ptl = pt_pool.tile([128, 2, 512], cd, tag="pt")
            nc.scalar.activation(ptl, sT_prev, mybir.ActivationFunctionType.Exp,
                                 scale=SCALE)
            for j in range(2):
                kt = (g - 1) * 2 + j
                mm(ps_aT, v[:, kt, :], ptl[:, j, :],
                   start=(kt == 0), stop=(kt == ST - 1))
            sT_prev = sT_next
        aT_sb = persist.tile([P + 1, R], f32, tag=out_name, name=out_name)
        nc.vector.tensor_copy(aT_sb, ps_aT)
        return aT_sb

    # ---------------------------------------------------------- masked branch
    amT = attention(kmT, vm, qmT, "amT")   # [65, R] unnormalized

    # normalize in [q, d] layout: A = A'[:, :64] / A'[:, 64]
    ps_a4 = kv([128, RT, P + 1])
    for qt in range(RT):
        tp(ps_a4[:, qt, :], amT[:, qt * 128:(qt + 1) * 128],
           ident_f32[0:P + 1, 0:P + 1])
    a_m = small.tile([128, RT, P], cd, tag="a_m")
    recip_m = small.tile([128, RT, 1], f32, tag="recip")
    for qt in range(RT):
        nc.vector.reciprocal(recip_m[:, qt, :], ps_a4[:, qt, P:P + 1])
        nc.vector.tensor_scalar_mul(a_m[:, qt, :], ps_a4[:, qt, 0:P],
                                    recip_m[:, qt, :])
    # back to A^T [64, R]
    ps_at2 = kv([P, R], cd)
    for qt in range(RT):
        tp(ps_at2[:, qt * 128:(qt + 1) * 128], a_m[:, qt, :], ident)
    amT_n = persist.tile([P, R], cd, tag="amT_n")
    nc.vector.tensor_copy(amT_n, ps_at2)

    # masked_mh^T [128, WC, R] = w_o_sum^T @ A
    mhT = persist.tile([128, WC, R], cd)
    for wc in range(WC):
        ps_mh = kv([128, R])
        mm(ps_mh, wosum[:, wc * 128:(wc + 1) * 128], amT_n)
        nc.vector.tensor_copy(mhT[:, wc, :], ps_mh)

    # ----------------------------------------------------------- cross branch
    qcT = proj_T(wqc, mhT, R, "qcT")
    acT = attention(kcT, vc, qcT, "acT")   # [65, R]; row 64 = denominators

    # denominators -> [q, 1] layout, reciprocal
    ps_s1 = kv([128, RT, 1])
    for qt in range(RT):
        tp(ps_s1[:, qt, :], acT[P:P + 1, qt * 128:(qt + 1) * 128],
           ident_f32[P:P + 1, P:P + 1])
    rs_c = small.tile([128, RT, 1], f32, tag="rs_c")
    for qt in range(RT):
        nc.vector.reciprocal(rs_c[:, qt, :], ps_s1[:, qt, :])

    if cd == f32:
        acT_cd = acT
    else:
        acT_cd = persist.tile([P + 1, R], cd, tag="acT_cd")
        nc.vector.tensor_copy(acT_cd, acT)

    # ----------------------------------------------- h = LN(mh_c + x) * g + b
    h_f32 = persist.tile([128, RT, W], f32)

    def layer_norm(dst, src):
        """dst = LN(src) * g + b  for [128, W] f32 tiles (may alias)."""
        stats = small.tile([128, 6], f32, tag="stats")
        nc.vector.bn_stats(stats, src)
        mv = small.tile([128, 2], f32, tag="mv")
        nc.vector.bn_aggr(mv, stats)
        nc.scalar.activation(mv[:, 1:2], mv[:, 1:2],
                             mybir.ActivationFunctionType.Sqrt,
                             bias=eps_t, scale=1.0)
        nc.vector.reciprocal(mv[:, 1:2], mv[:, 1:2])
        nc.vector.tensor_scalar(dst, src,
                                scalar1=mv[:, 0:1], scalar2=mv[:, 1:2],
                                op0=mybir.AluOpType.subtract,
                                op1=mybir.AluOpType.mult)
        nc.vector.tensor_mul(dst, dst, g_rep)
        nc.vector.tensor_add(dst, dst, b_rep)

    for qt in range(RT):
        ps_mhc = kv([128, W])
        mm(ps_mhc, acT_cd[0:P, qt * 128:(qt + 1) * 128], wosum)
        sum_sb = stream.tile([128, W], f32, tag="sum")
        nc.vector.tensor_scalar_mul(sum_sb, ps_mhc, rs_c[:, qt, :])
        nc.vector.tensor_add(sum_sb, sum_sb, xr_nat[:, qt, :])
        layer_norm(h_f32[:, qt, :], sum_sb)

    if cd == f32:
        h_cd = h_f32
    else:
        h_cd = persist.tile([128, RT, W], cd)
        nc.vector.tensor_copy(h_cd, h_f32)

    # h^T [128, WC, R]
    hT = persist.tile([128, WC, R], cd)
    for qt in range(RT):
        pst = big([128, WC, 128], cd)
        for wc in range(WC):
            tp(pst[:, wc, :], h_cd[:, qt, wc * 128:(wc + 1) * 128], ident)
        nc.vector.tensor_copy(hT[:, :, qt * 128:(qt + 1) * 128], pst)

    # ------------------------------------------------------------------- FFN
    w1_re = t["ffn_w1"].rearrange("(c p) f -> p c f", p=128)   # [128, WC, F]
    w2_re = t["ffn_w2"].rearrange("(c p) w -> p c w", p=128)   # [128, FC, W]
    y2_sb = persist.tile([128, RT, W], f32)

    for fg in range(FC // FG):
        w2g = wstream.tile([128, FG, W], f32, tag="w2g")
        nc.sync.dma_start(out=w2g, in_=w2_re[:, fg * FG:(fg + 1) * FG, :])
        if cd == f32:
            w2g_cd = w2g
        else:
            w2g_cd = wstream.tile([128, FG, W], cd, tag="w2g_cd")
            nc.vector.tensor_copy(w2g_cd, w2g)

        lT = pt_pool.tile([128, FG, R], cd, tag="lT")
        for fi in range(FG):
            fc = fg * FG + fi
            w1_sb = wstream.tile([128, WC, 128], f32, tag="w1_sb")
            nc.sync.dma_start(out=w1_sb, in_=w1_re[:, :, fc * 128:(fc + 1) * 128])
            if cd == f32:
                w1_cd = w1_sb
            else:
                w1_cd = wstream.tile([128, WC, 128], cd, tag="w1_cd")
                nc.vector.tensor_copy(w1_cd, w1_sb)
            ps_y1 = kv([128, R])
            for wc in range(WC):
                mm(ps_y1, w1_cd[:, wc, :], hT[:, wc, :],
                   start=(wc == 0), stop=(wc == WC - 1))
            # LeakyReLU(y1 + b1): parametric relu on the ACT engine
            nc.scalar.activation(lT[:, fi, :], ps_y1,
                                 mybir.ActivationFunctionType.Prelu,
                                 bias=b1_sb[:, fc:fc + 1], scale=1.0, alpha=LEAKY)
        for qt in range(RT):
            ps_p = kv([128, W])
            for fi in range(FG):
                mm(ps_p, lT[:, fi, qt * 128:(qt + 1) * 128], w2g_cd[:, fi, :],
                   start=(fi == 0), stop=(fi == FG - 1))
            if fg == 0:
                nc.vector.tensor_add(y2_sb[:, qt, :], ps_p, b2_rep)
            else:
                nc.vector.tensor_add(y2_sb[:, qt, :], y2_sb[:, qt, :], ps_p)

    # ------------------------------------------ out = LN(y2 + b2 + h) * g + b
    out_re = t["out"].rearrange("(q p) w -> q p w", p=128)
    for qt in range(RT):
        sum2 = stream.tile([128, W], f32, tag="sum")
        nc.vector.tensor_add(sum2, y2_sb[:, qt, :], h_f32[:, qt, :])
        layer_norm(sum2, sum2)
        nc.sync.dma_start(out=out_re[qt], in_=sum2)

    ctx.close()


_NC_CACHE = {}


def get_nc(mode=MODE):
    if mode not in _NC_CACHE:
        nc = build_nc(mode)
        nc.finalize()
        _NC_CACHE[mode] = nc
    return _NC_CACHE[mode]


def make_in_maps(inputs):
    x = np.ascontiguousarray(inputs["x"], dtype=np.float32)
    names = ["w_q_m", "w_k_m", "w_v_m", "w_q_c", "w_k_c", "w_v_c",
             "w_o", "ln_g", "ln_b", "ffn_w1", "ffn_b1", "ffn_w2", "ffn_b2"]
    shared = {n: np.ascontiguousarray(inputs[n], dtype=np.float32) for n in names}
    in_maps = []
    for c in range(NCORES):
        m = dict(shared)
        m["x_full"] = x
        m["x_rows"] = np.ascontiguousarray(x[c * R:(c + 1) * R])
        in_maps.append(m)
    return in_maps


def kernel(**inputs):
    in_maps = make_in_maps(inputs)
    nc = get_nc()
    res = run_bass_kernel_spmd(nc, in_maps, list(range(NCORES)))
    return np.concatenate([res.results[c]["out"] for c in range(NCORES)], axis=0)
